# revision 2
# baseline (speedup 1.0000x reference)
"""Trainium2 Bass kernel for nn_BinaryDense: out = x @ (sum_k sign(b_k)*a_k) + bias.

Shapes (hardcoded): x [4096,4096] f32, b [4,4096,4096] f32, a [4,4096] f32,
bias [4096] f32 -> out [4096,4096] f32.

Tensor-parallel over the output (units) dim across 8 NeuronCores; core c owns
O-columns [c*512, (c+1)*512).

Per core: one bf16 matmul x @ w (lhsT = host-pretransposed x^T tiles
stationary, w tiles moving, fp32 PSUM) with the weight w built on-chip:
w[:, oc] = sum_k copysign(a[k,oc], b[k,:,oc]).

v2 design (from ntff trace analysis of the 264.6us baseline):
  - b tiles stream on the Scalar-engine HWDGE ring, x^T tiles on the Sync
    ring: two independent hardware DMA queues, so the startup w-build is
    never starved behind x traffic.
  - w-build on DVE is one fused scalar_tensor_tensor ((b & 0x8000) | a,
    int16 lanes) + two bf16 pair-adds for the k-sum.
  - HAM warm-up: ~11 dummy matmuls issued during the DMA/build latency so
    the PE clock-gate opens before the first real matmul; a few filler
    matmuls inside the first m-block keep it open across build-chase stalls.
  - K-blocks [4,8,20]: the kb0/kb1 partial sums are evicted by the Scalar
    engine (PSUM->SBUF copy, bf16), combined + biased on GpSimd (SBUF only),
    so DVE does nothing but builds until the single final add per m-tile
    (out = psum + acc) in kb2. No DVE evict backlog -> no PE psum stalls.
  - Last m-block runs j-outer (per-m-tile complete groups) so the final
    adds + out-DMAs of 3 of its 4 m-tiles overlap the remaining matmuls
    (short tail). Out tiles leave on the Scalar ring (idle after b).

Host side only reshapes/casts/shards (no math): x^T bf16, b -> [I,K,O] bf16,
a/bias broadcast rows (bias bf16: adds ~0.3% of one bf16 ulp vs |out|~74).
"""

import os
import sys

if "/opt/trn_rl_repo" not in sys.path:
    sys.path.insert(0, "/opt/trn_rl_repo")

import numpy as np
import ml_dtypes

BF16 = ml_dtypes.bfloat16

B = 4096   # batch rows of x
I = 4096   # input dim (contraction)
O = 4096   # output dim (sharded)
K = 4      # binary bases
NCORES = 8
OC = O // NCORES   # 512 output cols per core
P = 128

KT = I // P        # 32 k-tiles (contraction)
MT = B // P        # 32 m-tiles (output rows)
M_BLOCK = 4        # m-tiles per psum block (4 banks, x2 parity = 8)
NMB = MT // M_BLOCK


def _build_program():
    import concourse.bass as bass
    import concourse.mybir as mybir
    from concourse import bacc
    from concourse.tile import TileContext

    K_BLOCKS = [int(s) for s in os.environ.get("BK_KBLOCKS", "4,8,20").split(",")]
    assert sum(K_BLOCKS) == KT
    N_DUM = int(os.environ.get("BK_DUMMIES", "11"))
    N_FILL = int(os.environ.get("BK_FILL", "6"))
    FUSED = os.environ.get("BK_FUSED", "1") == "1"
    UPFRONT_B = int(os.environ.get("BK_UPFRONT_B", "7"))
    DB_PER_UNIT = int(os.environ.get("BK_DB_PER_UNIT", "2"))

    nc = bacc.Bacc(None, target_bir_lowering=False)

    b_re = nc.declare_dram_parameter("b_re", [I, K * OC], mybir.dt.bfloat16, isOutput=False)
    a_b = nc.declare_dram_parameter("a_b", [P, K * OC], mybir.dt.bfloat16, isOutput=False)
    xT = nc.declare_dram_parameter("xT", [I, B], mybir.dt.bfloat16, isOutput=False)
    bias_b = nc.declare_dram_parameter("bias_b", [P, OC], mybir.dt.bfloat16, isOutput=False)
    out = nc.declare_dram_parameter("out", [B, OC], mybir.dt.float32, isOutput=True)

    # unit schedule: one unit per (k-block, m-block)
    units = []
    k0 = 0
    for kb, KB in enumerate(K_BLOCKS):
        kts = list(range(k0, k0 + KB))
        for mb in range(NMB):
            units.append((kb, mb, kts))
        k0 += KB
    NKB = len(K_BLOCKS)

    with TileContext(nc) as tc:
        with (
            tc.tile_pool(name="const", bufs=1) as const,
            tc.tile_pool(name="bpool", bufs=7) as bpool,
            tc.tile_pool(name="cpool", bufs=3) as cpool,
            tc.tile_pool(name="tpool", bufs=3) as tpool,
            tc.tile_pool(name="wpool", bufs=1) as wpool,
            tc.tile_pool(name="xpool", bufs=24) as xpool,
            tc.tile_pool(name="accA", bufs=1) as apool,
            tc.tile_pool(name="accB", bufs=1) as apool2,
            tc.tile_pool(name="opool", bufs=4) as opool,
            tc.tile_pool(name="psum", bufs=1, space="PSUM") as psum_pool,
        ):
            # ---- consts ----
            a_tile = const.tile([P, K * OC], mybir.dt.bfloat16)
            nc.sync.dma_start(out=a_tile[:], in_=a_b[:, :])
            bias_tile = const.tile([P, OC], mybir.dt.bfloat16)
            nc.gpsimd.dma_start(out=bias_tile[:], in_=bias_b[:, :])
            mask16 = const.tile([P, 1], mybir.dt.int16)
            nc.vector.memset(mask16[:], -32768)       # 0x8000
            mask32 = const.tile([P, 1], mybir.dt.int32)
            nc.vector.memset(mask32[:], -2147450880)  # 0x80008000
            dummy_w = const.tile([P, P], mybir.dt.bfloat16)
            nc.vector.memset(dummy_w[:], 0)
            dummy_rhs = const.tile([P, OC], mybir.dt.bfloat16)
            nc.vector.memset(dummy_rhs[:], 0)

            b_tiles, w_tiles, xt_tiles = {}, {}, {}
            acc_a = {m: apool.tile([P, OC], mybir.dt.bfloat16, name=f"acc_a_{m}")
                     for m in range(MT)}
            acc_b = {m: apool2.tile([P, OC], mybir.dt.bfloat16, name=f"acc_b_{m}")
                     for m in range(MT)}

            def emit_bdma(kt):
                bt = bpool.tile([P, K * OC], mybir.dt.bfloat16, name="b_tile")
                nc.scalar.dma_start(out=bt[:], in_=b_re[kt * P:(kt + 1) * P, :])
                b_tiles[kt] = bt

            def emit_build(kt):
                bt = b_tiles.pop(kt)
                c = cpool.tile([P, K * OC], mybir.dt.bfloat16, name="contrib")
                if FUSED:
                    nc.vector.scalar_tensor_tensor(
                        out=c.bitcast(mybir.dt.int16)[:],
                        in0=bt.bitcast(mybir.dt.int16)[:],
                        scalar=mask16[:, 0:1],
                        in1=a_tile.bitcast(mybir.dt.int16)[:],
                        op0=mybir.AluOpType.bitwise_and,
                        op1=mybir.AluOpType.bitwise_or,
                    )
                else:
                    nc.vector.tensor_scalar(
                        out=bt.bitcast(mybir.dt.int32)[:],
                        in0=bt.bitcast(mybir.dt.int32)[:],
                        scalar1=mask32[:, 0:1], scalar2=None,
                        op0=mybir.AluOpType.bitwise_and,
                    )
                    nc.vector.tensor_tensor(
                        out=c.bitcast(mybir.dt.int16)[:],
                        in0=bt.bitcast(mybir.dt.int16)[:],
                        in1=a_tile.bitcast(mybir.dt.int16)[:],
                        op=mybir.AluOpType.bitwise_or,
                    )
                t = tpool.tile([P, 2 * OC], mybir.dt.bfloat16, name="t_tile")
                nc.vector.tensor_tensor(
                    out=t[:], in0=c[:, 0:2 * OC], in1=c[:, 2 * OC:4 * OC],
                    op=mybir.AluOpType.add)
                w = wpool.tile([P, OC], mybir.dt.bfloat16, name=f"w_{kt}")
                nc.vector.tensor_tensor(
                    out=w[:], in0=t[:, 0:OC], in1=t[:, OC:2 * OC],
                    op=mybir.AluOpType.add)
                w_tiles[kt] = w

            def emit_xt(kt, mb):
                xt = xpool.tile([P, M_BLOCK * P], mybir.dt.bfloat16, name="xt")
                nc.sync.dma_start(
                    out=xt[:],
                    in_=xT[kt * P:(kt + 1) * P, mb * M_BLOCK * P:(mb + 1) * M_BLOCK * P])
                xt_tiles[(kt, mb)] = xt

            def emit_filler(n):
                for _ in range(n):
                    dps = psum_pool.tile([P, OC], mybir.dt.float32, name="ps_7")
                    nc.tensor.matmul(dps[:], dummy_w[:], dummy_rhs[:],
                                     start=True, stop=True)

            # ---- upfront: b doorbells, xt prefetch, HAM warm-up, first builds
            for kt in range(UPFRONT_B):
                emit_bdma(kt)
            for u in (0, 1):
                _, mb, kts = units[u]
                for kt in kts:
                    emit_xt(kt, mb)
            emit_filler(N_DUM)
            for kt in range(UPFRONT_B):
                emit_build(kt)
            bnext = UPFRONT_B

            # ---- main loop ----
            for u, (kb, mb, kts) in enumerate(units):
                parity = u % 2
                ps = {j: psum_pool.tile([P, OC], mybir.dt.float32,
                                        name=f"ps_{parity * 4 + j}")
                      for j in range(M_BLOCK)}
                nxt = units[u + 1] if u + 1 < len(units) else None
                last_unit = u == len(units) - 1

                if last_unit:
                    # j-outer: complete each m-tile's group, evict + store it
                    # while the next m-tile's matmuls still stream.
                    for j in range(M_BLOCK):
                        m = mb * M_BLOCK + j
                        for kt in kts:
                            nc.tensor.matmul(
                                ps[j][:],
                                xt_tiles[(kt, mb)][:, j * P:(j + 1) * P],
                                w_tiles[kt][:],
                                start=(kt == kts[0]), stop=(kt == kts[-1]))
                        o = opool.tile([P, OC], mybir.dt.float32, name="o_tile")
                        nc.vector.tensor_tensor(
                            out=o[:], in0=ps[j][:], in1=acc_a[m][:],
                            op=mybir.AluOpType.add)
                        nc.scalar.dma_start(out=out[m * P:(m + 1) * P, :], in_=o[:])
                    for kt in kts:
                        xt_tiles.pop((kt, mb))
                else:
                    done_pf = 0
                    for i, kt in enumerate(kts):
                        xt = xt_tiles.pop((kt, mb))
                        for j in range(M_BLOCK):
                            nc.tensor.matmul(
                                ps[j][:], xt[:, j * P:(j + 1) * P], w_tiles[kt][:],
                                start=(kt == kts[0]), stop=(kt == kts[-1]))
                        if u == 0 and i < len(kts) - 1:
                            emit_filler(N_FILL)
                        if u >= 1 and nxt is not None:
                            nkts = nxt[2]
                            tgt = min(((i + 1) * len(nkts) + len(kts) - 1) // len(kts),
                                      len(nkts))
                            while done_pf < tgt:
                                emit_xt(nkts[done_pf], nxt[1])
                                done_pf += 1

                    # evicts
                    for j in range(M_BLOCK):
                        m = mb * M_BLOCK + j
                        if kb == 0:
                            nc.scalar.copy(out=acc_a[m][:], in_=ps[j][:])
                        elif kb < NKB - 1:
                            nc.scalar.copy(out=acc_b[m][:], in_=ps[j][:])
                        else:
                            o = opool.tile([P, OC], mybir.dt.float32, name="o_tile")
                            nc.vector.tensor_tensor(
                                out=o[:], in0=ps[j][:], in1=acc_a[m][:],
                                op=mybir.AluOpType.add)
                            nc.scalar.dma_start(out=out[m * P:(m + 1) * P, :], in_=o[:])
                    if kb == NKB - 2:
                        # acc_a[m] += acc_b[m]; acc_a[m] += bias  (SBUF only)
                        for j in range(M_BLOCK):
                            m = mb * M_BLOCK + j
                            nc.gpsimd.tensor_tensor(
                                out=acc_a[m][:], in0=acc_a[m][:], in1=acc_b[m][:],
                                op=mybir.AluOpType.add)
                            nc.gpsimd.tensor_tensor(
                                out=acc_a[m][:], in0=acc_a[m][:], in1=bias_tile[:],
                                op=mybir.AluOpType.add)

                # paced b doorbells + builds
                while bnext <= min(UPFRONT_B - 1 + DB_PER_UNIT * (u + 1), KT - 1):
                    emit_bdma(bnext)
                    emit_build(bnext)
                    bnext += 1

    nc.compile()
    return nc


_NC_CACHE = None


def _get_program():
    global _NC_CACHE
    if _NC_CACHE is None:
        _NC_CACHE = _build_program()
    return _NC_CACHE


def prep_inputs(x, b, a, bias):
    """Host-side shard/cast/layout only. Returns per-core input maps."""
    x = np.asarray(x, dtype=np.float32)
    b = np.asarray(b, dtype=np.float32)
    a = np.asarray(a, dtype=np.float32)
    bias = np.asarray(bias, dtype=np.float32)
    xT16 = np.ascontiguousarray(x.T).astype(BF16)          # [I, B] bf16
    b_iko = np.transpose(b, (1, 0, 2)).astype(BF16)        # [I, K, O] bf16
    a16 = a.astype(BF16)                                    # [K, O]
    bias16 = bias.astype(BF16)

    in_maps = []
    for c in range(NCORES):
        sl = slice(c * OC, (c + 1) * OC)
        b_slice = np.ascontiguousarray(b_iko[:, :, sl]).reshape(I, K * OC)
        a_flat = np.ascontiguousarray(a16[:, sl]).reshape(1, K * OC)
        a_bcast = np.broadcast_to(a_flat, (P, K * OC)).copy()
        bias_bcast = np.broadcast_to(bias16[sl].reshape(1, OC), (P, OC)).copy()
        in_maps.append({
            "b_re": b_slice,
            "a_b": a_bcast,
            "xT": xT16,
            "bias_b": bias_bcast,
        })
    return in_maps


def run(in_maps, trace=False):
    from concourse.bass_utils import run_bass_kernel_spmd

    nc = _get_program()
    res = run_bass_kernel_spmd(nc, in_maps, list(range(NCORES)), trace=trace)
    return res


def kernel(x, b, a, bias):
    in_maps = prep_inputs(x, b, a, bias)
    res = run(in_maps)
    out = np.concatenate([res.results[c]["out"] for c in range(NCORES)], axis=1)
    return np.ascontiguousarray(out, dtype=np.float32)


if __name__ == "__main__":
    rng = np.random.default_rng(0)
    x = rng.standard_normal((B, I), dtype=np.float32)
    b = rng.standard_normal((K, I, O), dtype=np.float32)
    a = rng.random((K, O), dtype=np.float32)
    bias = rng.standard_normal(O, dtype=np.float32)
    out = kernel(x=x, b=b, a=a, bias=bias)
    w_eff = np.einsum('kio,ko->io', np.sign(b), a.astype(np.float64)).astype(np.float64)
    expected = x.astype(np.float64) @ w_eff + bias
    rel = np.linalg.norm(out - expected) / np.linalg.norm(expected)
    print(f"rel_err = {rel:.3e}")
